# revision 15
# baseline (speedup 1.0000x reference)
"""Trainium2 Bass kernel for a dense transformer block (pre-LN, causal MHA + FFN).

Sharding (8 NeuronCores): core c = 2*b + g handles sequence b (of B=4) and
half g (of 2): tensor-parallel attention over 8 of 16 heads (partial proj,
pairwise ReduceScatter over {2b, 2b+1}), then token-parallel LN2+FFN over
its 1024 of 2048 tokens.

Matmul strategy: fp8-e4m3 DoubleRow matmuls (0.5 cyc/row) everywhere except
the attention-score matmul (bf16). Accuracy-critical operands use 2-term fp8
splits (value + fp8 residual): Wproj, W1, W2 (host-side) and x2 (device-side,
Pool engine). LayerNorms are folded into the matmuls via augmented [1,2,*]
DoubleRow correction matmuls (colsum x (-mu), beta-dot x std) with per-column
rstd applied at eviction (Q, V) or inside exp (K, via activation scale).
Causal masking is done by accumulating ident20^T @ trilneg = -4800 onto the
diagonal stair blocks of S before exp (survives the ~1/32 exp scale).
"""
import numpy as np
import ml_dtypes
from contextlib import ExitStack

B, T, C = 4, 2048, 1024
H, HS = 16, 64
F = 4 * C
P = 128
EPS = 1e-5
NCT = C // P        # 8 c-tiles
NCP = NCT // 2      # 4 c-tile pairs
NPAIR = 4           # head-pairs per core
TL = T // 2         # 1024 local tokens
NFQ = 16            # f-tile pairs (FFN hidden 4096 = 32 tiles = 16 pairs)
GROUPS = [[0, 1], [2, 3], [4, 5], [6, 7]]
QC_ORDER = [0, 2, 1, 3]   # quarters 0,2 feed RS0; 1,3 feed RS1

E4 = ml_dtypes.float8_e4m3
BF = ml_dtypes.bfloat16

_CACHE = {}


def _build(with_collective=True):
    import concourse.tile as tile
    from concourse import bacc, mybir

    f32 = mybir.dt.float32
    f8 = mybir.dt.float8e4
    bf16 = mybir.dt.bfloat16
    AF = mybir.ActivationFunctionType
    OP = mybir.AluOpType
    PM = mybir.MatmulPerfMode

    nc = bacc.Bacc("TRN2", target_bir_lowering=False, debug=False, num_devices=8)

    # ---- DRAM I/O ----
    d_xa = nc.dram_tensor("xa", [P, NCP, 2, T], f8, kind="ExternalInput").ap()
    d_xsq = nc.dram_tensor("xsq", [P, NCP, 2, T], f8, kind="ExternalInput").ap()
    d_xres = nc.dram_tensor("xres", [NCT, P, TL], f32, kind="ExternalInput").ap()
    d_wqk = nc.dram_tensor("wqk", [P, NPAIR, NCP, 2, 256], f8,
                           kind="ExternalInput").ap()
    d_ccqk = nc.dram_tensor("ccqk", [NPAIR, 2, 256], f8, kind="ExternalInput").ap()
    d_wv = nc.dram_tensor("wv", [P, NPAIR, NCP, 2, P], f8,
                          kind="ExternalInput").ap()
    d_ccv = nc.dram_tensor("ccv", [NPAIR, 2, P], f8, kind="ExternalInput").ap()
    d_wpj = nc.dram_tensor("wpj", [P, 2, 2, 2, C], f8, kind="ExternalInput").ap()
    d_w1 = nc.dram_tensor("w1", [2, NCP, P, 2, F], f8, kind="ExternalInput").ap()
    d_cc1 = nc.dram_tensor("cc1", [2, F], f8, kind="ExternalInput").ap()
    d_w2 = nc.dram_tensor("w2", [2, NCT, P, NFQ, 2, P], f8,
                          kind="ExternalInput").ap()
    d_b2 = nc.dram_tensor("b2", [NCT, P], f32, kind="ExternalInput").ap()
    d_tril = nc.dram_tensor("trilneg", [P, P], f8, kind="ExternalInput").ap()
    d_id20 = nc.dram_tensor("ident20", [P, P], f8, kind="ExternalInput").ap()
    d_out = nc.dram_tensor("outT", [NCT, P, TL], f32, kind="ExternalOutput").ap()

    with tile.TileContext(nc) as tc, ExitStack() as ctx:
        dram = ctx.enter_context(tc.tile_pool(name="dram", bufs=1, space="DRAM"))
        sa_bounce = [dram.tile([2, NCT, P, 512], bf16, name=f"sab{r}")
                     for r in range(2)]
        sa_local = [dram.tile([NCT, P, 512], bf16, name=f"sal{r}")
                    for r in range(2)]
        x2_dram = dram.tile([NCT, P, TL], f32)

        # ---- persistent constants / inputs ----
        const = ctx.enter_context(tc.tile_pool(name="const", bufs=1))
        tril = const.tile([P, P], f8)
        nc.sync.dma_start(tril[:], d_tril[:])
        id20 = const.tile([P, P], f8)
        nc.sync.dma_start(id20[:], d_id20[:])
        ones16 = const.tile([P, 2, 16], f8)
        nc.vector.memset(ones16[:], 1.0)
        ones8 = ones16[:, :, 0:1]   # DR stationary needs 16B-aligned subtile step
        ones11 = const.tile([1, 1], f32)
        nc.vector.memset(ones11[:], 1.0)
        ebias = const.tile([P, 1], f32)
        nc.vector.memset(ebias[:], -2.0)

        # persistent row tiles (partition 0)
        rows_pool = ctx.enter_context(tc.tile_pool(name="rows", bufs=1))
        xaug = rows_pool.tile([1, 2, T], f8, tag="xaug")       # (-mu, std)
        xaug2 = rows_pool.tile([1, 2, TL], f8, tag="xaug2")

        bc_pool = ctx.enter_context(tc.tile_pool(name="bc", bufs=1))
        a1q = [bc_pool.tile([P, 512], f32, name=f"a1q{ch}", tag=f"a1q{ch}")
               for ch in range(4)]
        a2 = [bc_pool.tile([P, 512], f32, name=f"a2_{h}", tag=f"a2_{h}")
              for h in range(2)]
        rcol_pool = ctx.enter_context(tc.tile_pool(name="rcol", bufs=1))
        rvcol = rcol_pool.tile([P, 16], f32, tag="rvcol")      # rstd/32 by stripe
        # pin allocation order (pool frees must be LIFO vs first-use order)
        for t_ in (xaug, xaug2):
            nc.gpsimd.memset(t_[0:1, 0:1], 0.0)
        for t_ in a1q + a2 + [rvcol]:
            nc.gpsimd.memset(t_[:, 0:1], 0.0)

        x2q_pool = ctx.enter_context(tc.tile_pool(name="x2q", bufs=1))
        x2a = [x2q_pool.tile([P, 2, TL], f8, name=f"x2a{cp}", tag=f"x2a{cp}")
               for cp in range(NCP)]
        x2b = [x2q_pool.tile([P, 2, TL], f8, name=f"x2b{cp}", tag=f"x2b{cp}")
               for cp in range(NCP)]
        for t_ in x2a + x2b:
            nc.gpsimd.memset(t_[:, 0:1], 0.0)

        # attention-phase SBUF pools (closed after pattn; pinned below it)
        p3s = ExitStack()
        sasb_pool = p3s.enter_context(tc.tile_pool(name="sasb", bufs=4))
        end_pool = p3s.enter_context(tc.tile_pool(name="endp", bufs=4))
        rec_pool = p3s.enter_context(tc.tile_pool(name="recp", bufs=2))
        bcr_pool = p3s.enter_context(tc.tile_pool(name="bcrp", bufs=2))
        pin_sasb = sasb_pool.tile([P, 512], bf16, tag="sasb", name="pin_sasb")
        nc.gpsimd.memset(pin_sasb[:, 0:1], 0.0)
        pin_e = end_pool.tile([P, 2, 512], f8, tag="e", name="pin_e")
        nc.gpsimd.memset(pin_e[:, 0, 0:1], 0.0)
        pin_rec = rec_pool.tile([1, 512], f32, tag="rec", name="pin_rec")
        nc.gpsimd.memset(pin_rec[:, 0:1], 0.0)
        pin_bcr = bcr_pool.tile([64, 512], f32, tag="bcr", name="pin_bcr")
        nc.gpsimd.memset(pin_bcr[:, 0:1], 0.0)

        # attention working tiles (freed after proj)
        pattn = ExitStack()
        qk_pool = pattn.enter_context(tc.tile_pool(name="qk", bufs=1))
        qq = [qk_pool.tile([P, T], bf16, name=f"qq{p}", tag=f"qq{p}")
              for p in range(NPAIR)]
        kk = [qk_pool.tile([P, T], bf16, name=f"kk{p}", tag=f"kk{p}")
              for p in range(NPAIR)]
        for t_ in qq + kk:
            nc.gpsimd.memset(t_[:, 0:1], 0.0)
        va_pool = pattn.enter_context(tc.tile_pool(name="va", bufs=1))
        v_aug = {}
        for p in range(NPAIR):
            for sp in range(8):
                va = va_pool.tile([P, 2, 2, 72], f8, name=f"va{p}_{sp}",
                                  tag=f"va{p}_{sp}")
                nc.gpsimd.memset(va[:, :, :, 64:65], 1.0)
                v_aug[(p, sp)] = va
        ediag_pool = pattn.enter_context(tc.tile_pool(name="ediag", bufs=1))
        e_diag = {}
        for p in range(NPAIR):
            for hh in range(2):
                for di in range(2):
                    et = ediag_pool.tile([P, 2, 512], f8,
                                         name=f"ed{p}_{hh}_{di}",
                                         tag=f"ed{p}_{hh}_{di}")
                    for j in range(2):
                        z = (2 * di + j) * P
                        if z:
                            nc.gpsimd.memset(et[:, j, 0:z], 0.0)
                    e_diag[(p, hh, di)] = et
        attT_pool = pattn.enter_context(tc.tile_pool(name="attT", bufs=1))
        attT = [attT_pool.tile([P, 2, T], f8, name=f"attT{pp}", tag=f"attT{pp}")
                for pp in range(2)]
        for pp in range(2):
            nc.gpsimd.memset(attT[pp][:, :, 0:1], 0.0)  # pin alloc order

        pxin = ExitStack()
        xin_pool = pxin.enter_context(tc.tile_pool(name="xin", bufs=1))
        xa_t = xin_pool.tile([P, NCP, 2, T], f8, tag="xa")
        nc.sync.dma_start(xa_t[:], d_xa[:])
        xa = [xa_t[:, cp] for cp in range(NCP)]
        xsq_t = xin_pool.tile([P, NCP, 2, T], f8, tag="xsq")
        nc.sync.dma_start(xsq_t[:], d_xsq[:])
        xsq = [xsq_t[:, cp] for cp in range(NCP)]

        pwpj = ExitStack()
        wpj_pool = pwpj.enter_context(tc.tile_pool(name="wpjp", bufs=1))
        wpj_t = wpj_pool.tile([P, 2, 2, 2, C], f8, tag="wpj")
        nc.sync.dma_start(wpj_t[:], d_wpj[:])
        wpj = [wpj_t[:, sp_, pp] for sp_ in range(2) for pp in range(2)]

        pwqkv = ExitStack()
        wqkv_pool = pwqkv.enter_context(tc.tile_pool(name="wqkv", bufs=1))
        wqk_t = wqkv_pool.tile([P, NPAIR, NCP, 2, 256], f8, tag="wqk")
        nc.sync.dma_start(wqk_t[:], d_wqk[:])
        wqk = [wqk_t[:, p] for p in range(NPAIR)]
        ccqk_t = wqkv_pool.tile([1, NPAIR, 2, 256], f8, tag="ccqk")
        nc.sync.dma_start(ccqk_t[:], d_ccqk[:].unsqueeze(0))
        ccqk = [ccqk_t[:, p] for p in range(NPAIR)]
        wv_t = wqkv_pool.tile([P, NPAIR, NCP, 2, P], f8, tag="wv")
        nc.sync.dma_start(wv_t[:], d_wv[:])
        wv = [wv_t[:, p] for p in range(NPAIR)]
        ccv_t = wqkv_pool.tile([1, NPAIR, 2, P], f8, tag="ccv")
        nc.sync.dma_start(ccv_t[:], d_ccv[:].unsqueeze(0))
        ccv = [ccv_t[:, p] for p in range(NPAIR)]

        # =========== Phase 1: LN1 stats ===========
        p1 = ExitStack()
        st_ps1 = p1.enter_context(tc.tile_pool(name="stps1", bufs=2, space="PSUM"))
        row1_pool = p1.enter_context(tc.tile_pool(name="row1", bufs=6))
        for ch in range(4):
            sl = slice(ch * 512, (ch + 1) * 512)
            sx = st_ps1.tile([1, 512], f32, tag="sx")
            sq = st_ps1.tile([1, 512], f32, tag="sq")
            for cp in range(NCP):
                nc.tensor.matmul(sx[:], ones8, xa[cp][:, :, sl],
                                 start=(cp == 0), stop=(cp == NCP - 1),
                                 perf_mode=PM.DoubleRow)
                nc.tensor.matmul(sq[:], ones8, xsq[cp][:, :, sl],
                                 start=(cp == 0), stop=(cp == NCP - 1),
                                 perf_mode=PM.DoubleRow)
            # -mu (fp8 aug row) and f32 rows
            nc.scalar.activation(xaug[0:1, 0, sl], sx[:], AF.Copy, scale=-1.0 / C)
            mu = row1_pool.tile([1, 512], f32, tag="r")
            nc.scalar.activation(mu[:], sx[:], AF.Copy, scale=1.0 / C)
            ex2 = row1_pool.tile([1, 512], f32, tag="r")
            nc.scalar.activation(ex2[:], sq[:], AF.Copy, scale=1.0 / C)
            var = row1_pool.tile([1, 512], f32, tag="r")
            nc.vector.tensor_mul(var[:], mu[:], mu[:])
            nc.vector.scalar_tensor_tensor(var[:], ex2[:], EPS, var[:],
                                           OP.add, OP.subtract)
            std = row1_pool.tile([1, 512], f32, tag="r")
            nc.scalar.activation(std[:], var[:], AF.Sqrt)
            nc.vector.tensor_copy(xaug[0:1, 1, sl], std[:])
            rstd = row1_pool.tile([1, 512], f32, tag="r")
            nc.vector.reciprocal(rstd[:], std[:])
            r16 = row1_pool.tile([1, 512], f32, tag="r")
            nc.scalar.activation(r16[:], rstd[:], AF.Copy, scale=1.0 / 16)
            r32 = row1_pool.tile([1, 512], f32, tag="r")
            nc.scalar.activation(r32[:], rstd[:], AF.Copy, scale=1.0 / 32)
            nc.gpsimd.partition_broadcast(a1q[ch][:], r16[:])
            # rstd/32 per-stripe columns via mini PE transposes
            rc_ps = st_ps1.tile([P, 4], f32, tag="rc")
            for si in range(4):
                nc.tensor.transpose(rc_ps[:, si:si + 1],
                                    r32[:, si * P:(si + 1) * P], ones11[:])
            nc.vector.tensor_copy(rvcol[:, ch * 4:(ch + 1) * 4], rc_ps[:])
        p1.close()

        # =========== Phase 2a: Q,K (ch-major) ===========
        p2 = ExitStack()
        qkps = p2.enter_context(tc.tile_pool(name="qkps", bufs=2, space="PSUM"))
        for ch in range(4):
            sl = slice(ch * 512, (ch + 1) * 512)
            for p in range(NPAIR):
                q_ps = qkps.tile([P, 512], f32, tag="q")
                k_ps = qkps.tile([P, 512], f32, tag="k")
                for cp in range(NCP):
                    nc.tensor.matmul(q_ps[:], wqk[p][:, cp, :, 0:P],
                                     xa[cp][:, :, sl], start=(cp == 0),
                                     stop=False, perf_mode=PM.DoubleRow)
                    nc.tensor.matmul(k_ps[:], wqk[p][:, cp, :, P:256],
                                     xa[cp][:, :, sl], start=(cp == 0),
                                     stop=False, perf_mode=PM.DoubleRow)
                nc.tensor.matmul(q_ps[:], ccqk[p][:, :, 0:P], xaug[:, :, sl],
                                 start=False, stop=True, perf_mode=PM.DoubleRow)
                nc.tensor.matmul(k_ps[:], ccqk[p][:, :, P:256], xaug[:, :, sl],
                                 start=False, stop=True, perf_mode=PM.DoubleRow)
                nc.vector.tensor_mul(qq[p][:, sl], q_ps[:], a1q[ch][:])
                if (p + ch) % 2 == 0:
                    nc.scalar.copy(kk[p][:, sl], k_ps[:])
                else:
                    nc.vector.tensor_copy(kk[p][:, sl], k_ps[:])
        p2.close()

        # ===== Phases 2b+3+4: V^T + attention (qc-major) + proj + RS =====
        p3 = ExitStack()
        aps = p3.enter_context(tc.tile_pool(name="aps", bufs=2, space="PSUM"))

        def attn_unit(qc, p, hh):
            qsl = slice(qc * 512, (qc + 1) * 512)
            n_sp = 2 * (qc + 1)
            hsl = slice(hh * 64, (hh + 1) * 64)
            att = aps.tile([65, 512], f32, tag="att")
            pend = []  # (spi, et) with exp issued, PV pending

            def flush_pv(upto):
                while len(pend) > upto:
                    spi_, et_ = pend.pop(0)
                    nc.tensor.matmul(
                        att[:], v_aug[(p, spi_)][:, :, hh, 0:65], et_[:],
                        start=(spi_ == 0), stop=(spi_ == n_sp - 1),
                        perf_mode=PM.DoubleRow)

            for spi in range(n_sp):
                diag = (spi >= 2 * qc)
                if diag:
                    et = e_diag[(p, hh, spi - 2 * qc)]
                else:
                    et = end_pool.tile([P, 2, 512], f8, tag="e")
                stp = aps.tile([P, 2, 512], f32, tag="st")
                for j in range(2):
                    si = 2 * spi + j
                    ssl = slice(si * P, (si + 1) * P)
                    off = si - 4 * qc
                    nc.tensor.matmul(stp[:, j, :], kk[p][hsl, ssl],
                                     qq[p][hsl, qsl], start=True,
                                     stop=not diag)
                    if diag:
                        nc.tensor.matmul(stp[:, j, off * P:(off + 1) * P],
                                         id20[:], tril[:], start=False,
                                         stop=True, skip_group_check=True)
                if diag:
                    for j in range(2):
                        si = 2 * spi + j
                        off = si - 4 * qc
                        nc.scalar.activation(
                            et[0:P, j, off * P:512], stp[:, j, off * P:512],
                            AF.Exp, bias=ebias[:], scale=rvcol[:, si:si + 1])
                else:
                    nc.scalar.activation(
                        et[0:P, :, :], stp[:], AF.Exp, bias=ebias[:],
                        scale=rvcol[:, 2 * spi:2 * spi + 1])
                pend.append((spi, et))
                flush_pv(1)
            flush_pv(0)
            rec = rec_pool.tile([1, 512], f32, tag="rec")
            nc.vector.reciprocal(rec[:], att[64:65, :])
            bcr = bcr_pool.tile([64, 512], f32, tag="bcr")
            nc.gpsimd.partition_broadcast(bcr[:], rec[:])
            nc.vector.tensor_mul(attT[p // 2][hsl, p % 2, qsl],
                                 att[0:64, :], bcr[:])

        def proj_quarter(qc):
            qsl = slice(qc * 512, (qc + 1) * 512)
            r, fold = qc % 2, qc // 2
            for co in range(NCT):
                pp_ps = aps.tile([P, 512], f32, tag="vp")
                for pp in range(2):
                    for sp_ in range(2):
                        nc.tensor.matmul(
                            pp_ps[:],
                            wpj[sp_ * 2 + pp][:, :, co * P:(co + 1) * P],
                            attT[pp][:, :, qsl],
                            start=(pp == 0 and sp_ == 0),
                            stop=(pp == 1 and sp_ == 1),
                            perf_mode=PM.DoubleRow)
                sasb = sasb_pool.tile([P, 512], bf16, tag="sasb")
                nc.vector.tensor_copy(sasb[:], pp_ps[:])
                nc.sync.dma_start(sa_bounce[r][fold, co], sasb[:])
            if fold == 1:
                if with_collective:
                    nc.gpsimd.collective_compute(
                        "ReduceScatter", OP.add, replica_groups=GROUPS,
                        ins=[sa_bounce[r].opt()], outs=[sa_local[r].opt()])
                else:
                    nc.sync.dma_start(sa_local[r][:], sa_bounce[r][0])

        # V^T per pair, interleaved with qc0 attention (fills Act during QKV)
        for p in range(NPAIR):
            for st in range(16):
                ssl = slice(st * P, (st + 1) * P)
                v_ps = aps.tile([P, 512], f32, tag="vp")
                for cp in range(NCP):
                    nc.tensor.matmul(v_ps[:, 0:P], xa[cp][:, :, ssl],
                                     wv[p][:, cp], start=(cp == 0), stop=False,
                                     perf_mode=PM.DoubleRow)
                nc.tensor.matmul(v_ps[:, 0:P], xaug[:, :, ssl], ccv[p][:],
                                 start=False, stop=True, perf_mode=PM.DoubleRow)
                if st % 2 == 0:
                    nc.vector.tensor_scalar_mul(
                        v_aug[(p, st // 2)][:, st % 2, :, 0:64],
                        v_ps[:, 0:P].rearrange("a (b c) -> a b c", b=2),
                        rvcol[:, st:st + 1])
                else:
                    nc.scalar.activation(
                        v_aug[(p, st // 2)][:, st % 2, :, 0:64],
                        v_ps[:, 0:P].rearrange("a (b c) -> a b c", b=2),
                        AF.Copy, scale=rvcol[:, st:st + 1])
            for hh in range(2):
                attn_unit(0, p, hh)
        pwqkv.close()
        proj_quarter(0)

        for qc in [2, 1, 3]:
            for p in range(NPAIR):
                for hh in range(2):
                    attn_unit(qc, p, hh)
            proj_quarter(qc)
        p3.close()
        pwpj.close()
        pxin.close()
        pattn.close()
        p3s.close()

        # =========== Phase 5: x2 build (Pool) + LN2 stats ===========
        px2 = ExitStack()
        x2sq_pool = px2.enter_context(tc.tile_pool(name="x2sq", bufs=1))
        x2sq = [x2sq_pool.tile([P, 2, TL], f8, name=f"x2sq{cp}", tag=f"x2sq{cp}")
                for cp in range(NCP)]
        xres_pool = px2.enter_context(tc.tile_pool(name="xres", bufs=4))

        for r in range(2):
            lsl = slice(r * 512, (r + 1) * 512)
            for co in range(NCT):
                cp, j = co // 2, co % 2
                sal = xres_pool.tile([P, 512], bf16, tag="sal")
                nc.sync.dma_start(sal[:], sa_local[r][co])
                xre = xres_pool.tile([P, 512], f32, tag="xre")
                nc.sync.dma_start(xre[:], d_xres[co][:, lsl])
                x2w = xres_pool.tile([P, 512], f32, tag="x2w")
                nc.vector.scalar_tensor_tensor(x2w[:], sal[:], 1.0 / 32,
                                               xre[:], OP.mult, OP.add)
                nc.sync.dma_start(x2_dram[co][:, lsl], x2w[:])
                nc.scalar.copy(x2a[cp][:, j, lsl], x2w[:])
                nc.vector.scalar_tensor_tensor(x2b[cp][:, j, lsl], x2w[:], 0.0,
                                               x2a[cp][:, j, lsl],
                                               OP.add, OP.subtract)
                nc.gpsimd.tensor_mul(x2sq[cp][:, j, lsl], x2w[:], x2w[:])

        p5 = ExitStack()
        st_ps2 = p5.enter_context(tc.tile_pool(name="stps2", bufs=2, space="PSUM"))
        row2_pool = p5.enter_context(tc.tile_pool(name="row2", bufs=4))
        for h in range(2):
            lsl = slice(h * 512, (h + 1) * 512)
            sx = st_ps2.tile([1, 512], f32, tag="sx2")
            sq = st_ps2.tile([1, 512], f32, tag="sq2")
            for cp in range(NCP):
                nc.tensor.matmul(sx[:], ones8, x2a[cp][:, :, lsl],
                                 start=(cp == 0), stop=False,
                                 perf_mode=PM.DoubleRow)
                nc.tensor.matmul(sq[:], ones8, x2sq[cp][:, :, lsl],
                                 start=(cp == 0), stop=(cp == NCP - 1),
                                 perf_mode=PM.DoubleRow)
            for cp in range(NCP):
                nc.tensor.matmul(sx[:], ones8, x2b[cp][:, :, lsl],
                                 start=False, stop=(cp == NCP - 1),
                                 perf_mode=PM.DoubleRow)
            nc.scalar.activation(xaug2[0:1, 0, lsl], sx[:], AF.Copy,
                                 scale=-1.0 / C)
            mu = row2_pool.tile([1, 512], f32, tag="r")
            nc.scalar.activation(mu[:], sx[:], AF.Copy, scale=1.0 / C)
            ex2 = row2_pool.tile([1, 512], f32, tag="r")
            nc.scalar.activation(ex2[:], sq[:], AF.Copy, scale=1.0 / C)
            var = row2_pool.tile([1, 512], f32, tag="r")
            nc.vector.tensor_mul(var[:], mu[:], mu[:])
            nc.vector.scalar_tensor_tensor(var[:], ex2[:], EPS, var[:],
                                           OP.add, OP.subtract)
            std = row2_pool.tile([1, 512], f32, tag="r")
            nc.scalar.activation(std[:], var[:], AF.Sqrt)
            nc.vector.tensor_copy(xaug2[0:1, 1, lsl], std[:])
            rs2 = row2_pool.tile([1, 512], f32, tag="r")
            nc.vector.reciprocal(rs2[:], std[:])
            rs2s = row2_pool.tile([1, 512], f32, tag="r")
            nc.scalar.activation(rs2s[:], rs2[:], AF.Copy, scale=1.0 / 1024)
            nc.gpsimd.partition_broadcast(a2[h][:], rs2s[:])
        p5.close()
        px2.close()

        # =========== Phase 6: FFN ===========
        p6 = ExitStack()
        w1_pool = p6.enter_context(tc.tile_pool(name="w1", bufs=2))
        cc1_pool = p6.enter_context(tc.tile_pool(name="cc1", bufs=2))
        relu_pool = p6.enter_context(tc.tile_pool(name="relu", bufs=1))
        ffn_ps = p6.enter_context(tc.tile_pool(name="ffnps", bufs=4, space="PSUM"))
        relu = [relu_pool.tile([P, 2, TL], f8, name=f"rl{fq}", tag=f"rl{fq}")
                for fq in range(NFQ)]

        for fog in range(8):
            gsl = slice(fog * 512, (fog + 1) * 512)
            w1t = []
            for sp_ in range(2):
                for cp in range(NCP):
                    w = w1_pool.tile([P, 2, 512], f8, tag=f"w1_{sp_}{cp}")
                    nc.sync.dma_start(w[:], d_w1[sp_, cp][:, :, gsl])
                    w1t.append(w)
            cc1 = cc1_pool.tile([1, 2, 512], f8, tag="cc1")
            nc.sync.dma_start(cc1[:], d_cc1[:, gsl].unsqueeze(0))
            for fol in range(4):
                fo = fog * 4 + fol
                fsl = slice(fol * P, (fol + 1) * P)
                fq, fj = fo // 2, fo % 2
                for h in range(2):
                    lsl = slice(h * 512, (h + 1) * 512)
                    fp = ffn_ps.tile([P, 512], f32, tag="fp")
                    for cp in range(NCP):
                        nc.tensor.matmul(fp[:], w1t[cp][:, :, fsl],
                                         x2a[cp][:, :, lsl], start=(cp == 0),
                                         stop=False, perf_mode=PM.DoubleRow)
                    for cp in range(NCP):
                        nc.tensor.matmul(fp[:], w1t[NCP + cp][:, :, fsl],
                                         x2a[cp][:, :, lsl], start=False,
                                         stop=False, perf_mode=PM.DoubleRow)
                    for cp in range(NCP):
                        nc.tensor.matmul(fp[:], w1t[cp][:, :, fsl],
                                         x2b[cp][:, :, lsl], start=False,
                                         stop=False, perf_mode=PM.DoubleRow)
                    nc.tensor.matmul(fp[:], cc1[:, :, fsl], xaug2[:, :, lsl],
                                     start=False, stop=True,
                                     perf_mode=PM.DoubleRow)
                    if fo % 2 == 0:
                        nc.scalar.activation(relu[fq][:, fj, lsl], fp[:], AF.Relu)
                    else:
                        nc.vector.tensor_scalar_max(relu[fq][:, fj, lsl], fp[:],
                                                    0.0)

        w2_pool = p6.enter_context(tc.tile_pool(name="w2", bufs=4))
        out_pool = p6.enter_context(tc.tile_pool(name="outsb", bufs=4))
        b2_pool = p6.enter_context(tc.tile_pool(name="b2p", bufs=1))
        b2col = [b2_pool.tile([P, 1], f32, name=f"b2c{co}", tag=f"b2c{co}")
                 for co in range(NCT)]
        for co in range(NCT):
            nc.sync.dma_start(b2col[co][:], d_b2[co].unsqueeze(1))
        for co in range(NCT):
            w2a = w2_pool.tile([P, NFQ, 2, P], f8, tag="w2t")
            nc.sync.dma_start(w2a[:], d_w2[0, co])
            w2b = w2_pool.tile([P, NFQ, 2, P], f8, tag="w2t")
            nc.sync.dma_start(w2b[:], d_w2[1, co])
            for h in range(2):
                lsl = slice(h * 512, (h + 1) * 512)
                fp = ffn_ps.tile([P, 512], f32, tag="fp2")
                for fq in range(NFQ):
                    nc.tensor.matmul(fp[:], w2a[:, fq], relu[fq][:, :, lsl],
                                     start=(fq == 0), stop=False,
                                     perf_mode=PM.DoubleRow)
                for fq in range(NFQ):
                    nc.tensor.matmul(fp[:], w2b[:, fq], relu[fq][:, :, lsl],
                                     start=False, stop=(fq == NFQ - 1),
                                     perf_mode=PM.DoubleRow)
                x2c = out_pool.tile([P, 512], f32, tag="x2c")
                nc.sync.dma_start(x2c[:], x2_dram[co][:, lsl])
                tmp = out_pool.tile([P, 512], f32, tag="tmp")
                nc.vector.tensor_mul(tmp[:], fp[:], a2[h][:])
                osb = out_pool.tile([P, 512], f32, tag="osb")
                nc.vector.scalar_tensor_tensor(osb[:], tmp[:], b2col[co][:],
                                               x2c[:], OP.add, OP.add)
                nc.sync.dma_start(d_out[co][:, lsl], osb[:])
        p6.close()

    nc.compile()
    return nc


def _q8(v):
    return np.asarray(v, np.float32).astype(E4)


def _prep_inputs(x, Wq, Wk, Wv, Wproj, bproj, W1, b1, W2, b2, g1, beta1, g2,
                 beta2):
    f32 = np.float32
    scale = HS ** -0.5
    x = np.asarray(x, f32)
    Wq = np.asarray(Wq, f32); Wk = np.asarray(Wk, f32); Wv = np.asarray(Wv, f32)
    Wproj = np.asarray(Wproj, f32); bproj = np.asarray(bproj, f32)
    W1 = np.asarray(W1, f32); b1 = np.asarray(b1, f32)
    W2 = np.asarray(W2, f32); b2 = np.asarray(b2, f32)
    g1 = np.asarray(g1, f32); beta1 = np.asarray(beta1, f32)
    g2 = np.asarray(g2, f32); beta2 = np.asarray(beta2, f32)

    # ---- shared (g-independent) weights ----
    w1s = (g2[:, None] * W1) * 32.0                       # [C, F]
    w1a = _q8(w1s)
    w1b = _q8(w1s - w1a.astype(f32))
    w1_pack = np.stack([w1a.reshape(NCP, 2, P, F).transpose(0, 2, 1, 3),
                        w1b.reshape(NCP, 2, P, F).transpose(0, 2, 1, 3)])
    cc1 = np.empty((2, F), f32)
    cc1[0] = (w1a.astype(f32) + w1b.astype(f32)).sum(0)
    cc1[1] = (b1 + beta2 @ W1) * 32.0
    cc1 = _q8(cc1)

    w2s = W2 * 32.0                                        # [F, C]
    w2a = _q8(w2s)
    w2b = _q8(w2s - w2a.astype(f32))
    # [2, NCT, P, NFQ, 2, P]: w2[s, co, p, fq, j, cc] = w2s[(2fq+j)*128+p, co*128+cc]
    def pack_w2(w):
        return np.ascontiguousarray(
            w.reshape(NFQ, 2, P, NCT, P).transpose(3, 2, 0, 1, 4))
    w2_pack = np.stack([pack_w2(w2a), pack_w2(w2b)])

    b2r = b2.reshape(NCT, P)
    kp = np.arange(P)[:, None]
    lq = np.arange(P)[None, :]
    trilneg = (-240.0 * (lq < kp)).astype(E4)
    ident20 = (20.0 * np.eye(P)).astype(E4)

    # ---- per-batch x ----
    xa_b, xsq_b, xresT_b = [], [], []
    for b in range(B):
        xT = np.ascontiguousarray(x[b].T)                  # [C, T]
        xq = _q8(xT)
        xa_b.append(np.ascontiguousarray(
            xq.reshape(NCP, 2, P, T).transpose(2, 0, 1, 3)))
        xsq_b.append(np.ascontiguousarray(
            _q8(xq.astype(f32) ** 2).reshape(NCP, 2, P, T).transpose(2, 0, 1, 3)))
        xresT_b.append(xT + bproj[:, None])                # bproj folded in

    # ---- per-group attention weights ----
    per_g = {}
    for g in range(2):
        wqk = np.empty((NPAIR, P, NCP, 2, 256), E4)
        ccqk = np.empty((NPAIR, 2, 256), f32)
        wv_ = np.empty((NPAIR, P, NCP, 2, P), E4)
        ccv = np.empty((NPAIR, 2, P), f32)
        for p in range(NPAIR):
            hA, hB = g * 8 + 2 * p, g * 8 + 2 * p + 1
            for (Wfull, scl, col) in ((Wq, scale * 16.0, slice(0, P)),
                                      (Wk, 32.0, slice(P, 256))):
                wt = np.concatenate([Wfull[hA], Wfull[hB]], axis=1) * scl  # [C,128]
                wq8 = _q8(g1[:, None] * wt)
                wqk[p, :, :, :, col] = wq8.reshape(NCP, 2, P, P).transpose(
                    2, 0, 1, 3)
                ccqk[p, 0, col] = wq8.astype(f32).sum(0)
                ccqk[p, 1, col] = beta1 @ wt
            wt = np.concatenate([Wv[hA], Wv[hB]], axis=1) * 32.0
            wq8 = _q8(g1[:, None] * wt)
            wv_[p] = wq8.reshape(NCP, 2, P, P).transpose(2, 0, 1, 3)
            ccv[p, 0] = wq8.astype(f32).sum(0)
            ccv[p, 1] = beta1 @ wt
        # wpj [2(split), 2(pp), P, 2(j), C]: wps[pp, j, f, c] -> [pp, f, j, c]
        wps = np.ascontiguousarray(
            Wproj[g * 512:(g + 1) * 512] * 32.0).reshape(2, 2, P, C)
        wpa = _q8(wps)
        wpb = _q8(wps - wpa.astype(f32))
        wpj = np.stack([wpa.transpose(0, 2, 1, 3), wpb.transpose(0, 2, 1, 3)])
        per_g[g] = dict(wqk=np.ascontiguousarray(wqk.transpose(1, 0, 2, 3, 4)),
                        ccqk=_q8(ccqk),
                        wv=np.ascontiguousarray(wv_.transpose(1, 0, 2, 3, 4)),
                        ccv=_q8(ccv),
                        wpj=np.ascontiguousarray(wpj.transpose(2, 0, 1, 3, 4)))

    in_maps = []
    for c in range(8):
        b, g = c // 2, c % 2
        m = dict(
            xa=xa_b[b], xsq=xsq_b[b],
            xres=np.ascontiguousarray(
                xresT_b[b][:, g * TL:(g + 1) * TL].reshape(NCT, P, TL)),
            w1=w1_pack, cc1=cc1, w2=w2_pack, b2=b2r,
            trilneg=trilneg, ident20=ident20,
        )
        m.update(per_g[g])
        in_maps.append(m)
    return in_maps


def kernel(**inputs):
    from concourse.bass_utils import run_bass_kernel_spmd

    if "nc" not in _CACHE:
        _CACHE["nc"] = _build(with_collective=True)
    nc = _CACHE["nc"]
    in_maps = _prep_inputs(**inputs)
    res = None
    last_err = None
    for _attempt in range(3):
        try:
            res = run_bass_kernel_spmd(nc, in_maps, list(range(8)))
            break
        except Exception as e:  # transient runtime/tunnel hiccups
            last_err = e
            import time
            time.sleep(10)
    if res is None:
        raise last_err
    out = np.empty((B, T, C), np.float32)
    for c in range(8):
        b, g = c // 2, c % 2
        outT = res.results[c]["outT"].reshape(C, TL)
        out[b, g * TL:(g + 1) * TL, :] = outT.T
    return out
